# revision 1
# baseline (speedup 1.0000x reference)
"""M2MRF module as a two-GEMM chained Bass kernel on 8 TRN2 NeuronCores.

Math (per batch b of 4):
    cols = unfold(x[b], k=4, s=4)            # [1024, 16384]
    y1   = W1 @ cols + b1                    # [1024, 16384]
    y2   = W2 @ y1 + b2                      # [256, 16384]
    out[b] = fold(y2, k=2, s=2)              # [64, 256, 256]

Sharding: 8 cores = 4 batches x 2 L-halves (L = 16384 patch positions).
Each core runs GEMM1 (1024x1024x8192) + GEMM2 (256x1024x8192) in bf16
with fp32 PSUM accumulation. Unfold/fold are pure data-movement and run
on the host; the device sees contiguous [K, L] operands resident in SBUF.
"""
import sys

sys.path.insert(0, "/opt/trn_rl_repo")

import numpy as np
import ml_dtypes

import concourse.bass as bass
import concourse.bacc as bacc
import concourse.mybir as mybir
import concourse.tile as tile
from concourse.bass_utils import run_bass_kernel_spmd

P = 128
NT = 512            # free-dim tile (one PSUM bank of fp32)
LSH = 8192          # L per core
NTILES = LSH // NT  # 16
KC = 8              # 1024 / 128 contraction chunks
FC = 1024
COUT = 256

_BF16 = ml_dtypes.bfloat16


def _build_nc(ntiles=NTILES):
    nc = bacc.Bacc("TRN2", target_bir_lowering=False)
    xc_dram = [
        nc.dram_tensor(f"xc{k}", [P, LSH], mybir.dt.bfloat16, kind="ExternalInput")
        for k in range(KC)
    ]
    w1_dram = nc.dram_tensor("w1t", [KC, P, FC], mybir.dt.bfloat16, kind="ExternalInput")  # [m, p, k*128+j]
    w2_dram = nc.dram_tensor("w2t", [KC, P, COUT], mybir.dt.bfloat16, kind="ExternalInput")
    y2_dram = nc.dram_tensor("y2", [2, P, LSH], mybir.dt.float32, kind="ExternalOutput")

    with tile.TileContext(nc) as tc:
        with (
            tc.tile_pool(name="resident", bufs=1) as res,
            tc.tile_pool(name="work", bufs=2) as work,
            tc.tile_pool(name="outp", bufs=3) as outp,
            tc.tile_pool(name="ps1", bufs=4, space="PSUM") as ps1,
            tc.tile_pool(name="ps2", bufs=2, space="PSUM") as ps2,
        ):
            w1_sb = res.tile([P, KC, FC], mybir.dt.bfloat16, tag="w1")
            w2_sb = res.tile([P, KC, COUT], mybir.dt.bfloat16, tag="w2")
            xc_sb = [
                res.tile([P, LSH], mybir.dt.bfloat16, tag=f"xc{k}", name=f"xc{k}")
                for k in range(KC)
            ]
            # Issue order tracks first use: the opening m-group of tile 0 needs
            # only W1's m=0 slice plus the head slice of every x chunk.
            nc.sync.dma_start(w1_sb[:, 0, :], w1_dram.ap()[0])
            hsl = slice(0, LSH // 8)
            for k in range(KC):
                nc.sync.dma_start(xc_sb[k][:, hsl], xc_dram[k].ap()[:, hsl])
            for m in range(1, KC):
                nc.sync.dma_start(w1_sb[:, m, :], w1_dram.ap()[m])
            nc.sync.dma_start(w2_sb[:], w2_dram.ap().rearrange("k p m -> p k m"))
            for h in range(1, 8):
                sl = slice(h * (LSH // 8), (h + 1) * (LSH // 8))
                for k in range(KC):
                    nc.sync.dma_start(xc_sb[k][:, sl], xc_dram[k].ap()[:, sl])

            for nt in range(ntiles):
                nsl = slice(nt * NT, (nt + 1) * NT)
                y1_sb = work.tile([P, KC, NT], mybir.dt.bfloat16, tag="y1")
                # GEMM1: y1[m,:] = sum_k W1T[k,:,m]^T @ xc[k][:, nsl]
                for m in range(KC):
                    pt = ps1.tile([P, NT], mybir.dt.float32, tag="ps1")
                    for k in range(KC):
                        nc.tensor.matmul(
                            pt[:],
                            w1_sb[:, m, k * P:(k + 1) * P],
                            xc_sb[k][:, nsl],
                            start=(k == 0),
                            stop=(k == KC - 1),
                        )
                    nc.vector.tensor_copy(y1_sb[:, m, :], pt[:])
                # GEMM2: y2[m2,:] = sum_k W2T[k,:,m2]^T @ y1[k,:]
                o_sb = outp.tile([P, 2, NT], mybir.dt.float32, tag="o")
                for m2 in range(2):
                    pt2 = ps2.tile([P, NT], mybir.dt.float32, tag="ps2")
                    for k in range(KC):
                        nc.tensor.matmul(
                            pt2[:],
                            w2_sb[:, k, m2 * P:(m2 + 1) * P],
                            y1_sb[:, k, :],
                            start=(k == 0),
                            stop=(k == KC - 1),
                        )
                    nc.any.tensor_copy(out=o_sb[:, m2, :], in_=pt2[:])
                    nc.sync.dma_start(y2_dram.ap()[m2, :, nsl], o_sb[:, m2, :])

    nc.finalize()
    return nc


_NC_CACHE = None


def kernel(x, W1, b1, W2, b2):
    global _NC_CACHE
    x = np.asarray(x)
    W1, b1 = np.asarray(W1), np.asarray(b1)
    W2, b2 = np.asarray(W2), np.asarray(b2)
    n, c, h, w = x.shape  # 4, 64, 512, 512

    # ---- host unfold: cols[b, c*16+kh*4+kw, ph*128+pw] = x[b,c,ph*4+kh,pw*4+kw]
    xb = x.astype(_BF16)
    cols = xb.reshape(n, c, 128, 4, 128, 4).transpose(0, 1, 3, 5, 2, 4)
    cols = np.ascontiguousarray(cols).reshape(n, 1024, 16384)

    w1t = np.ascontiguousarray(
        W1.astype(_BF16).reshape(KC, P, KC, P).transpose(0, 3, 2, 1)
    ).reshape(KC, P, FC)
    w2t = np.ascontiguousarray(W2.T.astype(_BF16)).reshape(KC, P, COUT)

    if _NC_CACHE is None:
        _NC_CACHE = _build_nc()
    nc = _NC_CACHE

    in_maps = []
    for core in range(8):
        b, half = core // 2, core % 2
        xc = np.ascontiguousarray(
            cols[b, :, half * LSH:(half + 1) * LSH]
        ).reshape(KC, P, LSH)
        m = {f"xc{k}": xc[k] for k in range(KC)}
        m["w1t"] = w1t
        m["w2t"] = w2t
        in_maps.append(m)

    res = run_bass_kernel_spmd(nc, in_maps, core_ids=list(range(8)))

    # ---- gather + fold on host
    y2 = np.empty((n, COUT, 16384), dtype=np.float32)
    for core in range(8):
        b, half = core // 2, core % 2
        y2[b, :, half * LSH:(half + 1) * LSH] = (
            res.results[core]["y2"].reshape(COUT, LSH)
        )

    # bias epilogue (b1/b2 are zeros in this problem; exact otherwise)
    v = W2.astype(np.float64) @ b1.astype(np.float64) + b2.astype(np.float64)
    if np.any(v):
        y2 += v.astype(np.float32)[None, :, None]

    out = y2.reshape(n, c, 2, 2, 128, 128).transpose(0, 1, 4, 2, 5, 3)
    return np.ascontiguousarray(out).reshape(n, c, 256, 256)



# revision 3
# speedup vs baseline: 2.5999x; 2.5999x over previous
"""M2MRF module as a single fused GEMM on 8 TRN2 NeuronCores.

The reference chains two 1x1 convs with no nonlinearity between them:
    y2 = W2 @ (W1 @ cols + b1) + b2 = (W2 @ W1) @ cols + (W2 @ b1 + b2)
so the device only runs one GEMM with W_eff = W2 @ W1 (precomputed on
host in fp64):
    cols = unfold(x[b], k=4, s=4)            # [1024, 16384]
    y    = W_eff @ cols                      # [256, 16384]
    out[b] = fold(y, k=2, s=2)               # [64, 256, 256]

Sharding: 8 cores = 4 batches x 2 L-halves (L = 16384 patch positions).
Each core runs GEMM (256x1024x8192) in bf16 with fp32 PSUM accumulation
and writes a bf16 result. Unfold/fold are pure data movement on host.
"""
import sys

sys.path.insert(0, "/opt/trn_rl_repo")

import numpy as np
import ml_dtypes

import concourse.bass as bass
import concourse.bacc as bacc
import concourse.mybir as mybir
import concourse.tile as tile
from concourse.bass_utils import run_bass_kernel_spmd

P = 128
NT = 512            # free-dim tile (one PSUM bank of fp32)
LSH = 8192          # L per core
NTILES = LSH // NT  # 16
KC = 8              # 1024 / 128 contraction chunks
COUT = 256

_BF16 = ml_dtypes.bfloat16


def _build_nc(ntiles=NTILES):
    nc = bacc.Bacc("TRN2", target_bir_lowering=False)
    xc_dram = nc.dram_tensor("xc", [KC, P, LSH], mybir.dt.bfloat16, kind="ExternalInput")
    w_dram = nc.dram_tensor("wt", [KC, P, COUT], mybir.dt.bfloat16, kind="ExternalInput")
    y_dram = nc.dram_tensor("y", [2, P, LSH], mybir.dt.bfloat16, kind="ExternalOutput")

    with tile.TileContext(nc) as tc:
        with (
            tc.tile_pool(name="resident", bufs=1) as res,
            tc.tile_pool(name="outp", bufs=4) as outp,
            tc.tile_pool(name="ps", bufs=4, space="PSUM") as ps,
        ):
            w_sb = res.tile([P, KC, COUT], mybir.dt.bfloat16, tag="w")
            xc_sb = [
                res.tile([P, LSH], mybir.dt.bfloat16, tag=f"xc{k}", name=f"xc{k}")
                for k in range(KC)
            ]
            # Weights first (small, needed by every tile), then x slices in
            # n-tile order so compute can start as soon as slice 0 lands.
            nc.sync.dma_start(w_sb[:], w_dram.ap().rearrange("k p m -> p k m"))
            for nt in range(ntiles):
                nsl = slice(nt * NT, (nt + 1) * NT)
                for k in range(KC):
                    nc.sync.dma_start(xc_sb[k][:, nsl], xc_dram.ap()[k][:, nsl])

            for nt in range(ntiles):
                nsl = slice(nt * NT, (nt + 1) * NT)
                o_sb = outp.tile([P, 2, NT], mybir.dt.bfloat16, tag="o")
                for m in range(2):
                    pt = ps.tile([P, NT], mybir.dt.float32, tag="ps")
                    for k in range(KC):
                        nc.tensor.matmul(
                            pt[:],
                            w_sb[:, k, m * P:(m + 1) * P],
                            xc_sb[k][:, nsl],
                            start=(k == 0),
                            stop=(k == KC - 1),
                        )
                    nc.any.tensor_copy(out=o_sb[:, m, :], in_=pt[:])
                    nc.scalar.dma_start(y_dram.ap()[m][:, nsl], o_sb[:, m, :])

    nc.finalize()
    return nc


_NC_CACHE = None


def kernel(x, W1, b1, W2, b2):
    global _NC_CACHE
    x = np.asarray(x)
    W1, b1 = np.asarray(W1), np.asarray(b1)
    W2, b2 = np.asarray(W2), np.asarray(b2)
    n, c, h, w = x.shape  # 4, 64, 512, 512

    # ---- host unfold: cols[b, c*16+kh*4+kw, ph*128+pw] = x[b,c,ph*4+kh,pw*4+kw]
    xb = x.astype(_BF16)
    cols = xb.reshape(n, c, 128, 4, 128, 4).transpose(0, 1, 3, 5, 2, 4)
    cols = np.ascontiguousarray(cols).reshape(n, 1024, 16384)

    # ---- fold the two GEMMs into one
    Weff = (W2.astype(np.float64) @ W1.astype(np.float64)).astype(np.float32)
    wt = np.ascontiguousarray(Weff.T.astype(_BF16)).reshape(KC, P, COUT)

    if _NC_CACHE is None:
        _NC_CACHE = _build_nc()
    nc = _NC_CACHE

    in_maps = []
    for core in range(8):
        b, half = core // 2, core % 2
        xc = np.ascontiguousarray(
            cols[b, :, half * LSH:(half + 1) * LSH]
        ).reshape(KC, P, LSH)
        in_maps.append({"xc": xc, "wt": wt})

    res = run_bass_kernel_spmd(nc, in_maps, core_ids=list(range(8)))

    # ---- gather + fold on host
    y2 = np.empty((n, COUT, 16384), dtype=np.float32)
    for core in range(8):
        b, half = core // 2, core % 2
        y2[b, :, half * LSH:(half + 1) * LSH] = (
            res.results[core]["y"].reshape(COUT, LSH).astype(np.float32)
        )

    # bias epilogue (b1/b2 are zeros in this problem; exact otherwise)
    v = W2.astype(np.float64) @ b1.astype(np.float64) + b2.astype(np.float64)
    if np.any(v):
        y2 += v.astype(np.float32)[None, :, None]

    out = y2.reshape(n, c, 2, 2, 128, 128).transpose(0, 1, 4, 2, 5, 3)
    return np.ascontiguousarray(out).reshape(n, c, 256, 256)


# revision 11
# speedup vs baseline: 4.7730x; 1.8358x over previous
"""M2MRF module as a single fused mixed-precision GEMM on 8 TRN2 NeuronCores.

The reference chains two 1x1 convs with no nonlinearity between them, so
    y2 = W2 @ (W1 @ cols + b1) + b2 = W_eff @ cols + const,
with W_eff = W2 @ W1 precomputed on host (fp64). The device runs the
single GEMM y = W_eff @ cols per batch shard:
    cols = unfold(x[b], k=4, s=4)            # [1024, 16384]
    y    = W_eff @ cols                      # [256, 16384]
    out[b] = fold(y, k=2, s=2)               # [64, 256, 256]

Sharding: 8 cores = 4 batches x 2 L-halves (LSH = 8192 columns/core).

Precision scheme (all products share scale S = 128, one PSUM group):
 - channels 0..511  (e3 part): x3 = e3m4(2x), w3 = e3m4(64W); plain
   fp8 matmuls (1 cycle/row).
 - channels 512..1023 (e4 part): xa = e4m3(2x), xb = e4m3(2x - xa),
   wh = e4m3(64W), wl = e4m3(64W - wh). DoubleRow fp8 matmuls
   (0.5 cycle/row) compute wh@xa (bulk, chunk pairs) and the
   correction wl@xa + wh@xb (paired per chunk), dropping only the
   tiny wl@xb term.
PSUM accumulates everything in fp32; result is written as bf16 of
128*y and rescaled on host. Measured rel err vs the fp32 reference is
~1.2e-2 (gate: 2e-2).
"""
import sys

sys.path.insert(0, "/opt/trn_rl_repo")

import numpy as np
import ml_dtypes

import concourse.bass as bass
import concourse.bacc as bacc
import concourse.mybir as mybir
import concourse.tile as tile
from concourse.bass_utils import run_bass_kernel_spmd

P = 128
NT = 512            # free-dim tile (one PSUM bank of fp32)
LSH = 8192          # L per core
NTILES = LSH // NT  # 16
K3C = 4             # e3m4 chunks (channels 0..511)
K4C = 4             # e4m3 chunks (channels 512..1023)
COUT = 256

_BF16 = ml_dtypes.bfloat16
_E3 = ml_dtypes.float8_e3m4
_E4 = ml_dtypes.float8_e4m3
_DR = mybir.MatmulPerfMode.DoubleRow


def _build_nc(ntiles=NTILES):
    nc = bacc.Bacc("TRN2", target_bir_lowering=False)
    x3_dram = nc.dram_tensor("x3", [K3C, P, LSH], mybir.dt.float8e3, kind="ExternalInput")
    xab_dram = nc.dram_tensor("xab", [K4C, 2, P, LSH], mybir.dt.float8e4, kind="ExternalInput")
    w3_dram = nc.dram_tensor("w3", [K3C, P, COUT], mybir.dt.float8e3, kind="ExternalInput")
    wlh_dram = nc.dram_tensor("wlh", [K4C, 2, P, COUT], mybir.dt.float8e4, kind="ExternalInput")
    y_dram = nc.dram_tensor("y", [2, P, LSH], mybir.dt.bfloat16, kind="ExternalOutput")

    with tile.TileContext(nc) as tc:
        with (
            tc.tile_pool(name="resident", bufs=1) as res,
            tc.tile_pool(name="outp", bufs=4) as outp,
            tc.tile_pool(name="ps", bufs=4, space="PSUM") as ps,
        ):
            w3_sb = res.tile([P, K3C, COUT], mybir.dt.float8e3, tag="w3")
            wlh_sb = res.tile([P, K4C, 2, COUT], mybir.dt.float8e4, tag="wlh")
            x3_sb = res.tile([P, K3C, LSH], mybir.dt.float8e3, tag="x3")
            xab_sb = res.tile([P, K4C, 2, LSH], mybir.dt.float8e4, tag="xab")
            # DMA order tracks first use: w3 + tile-0 x3 feed the opening e3
            # matmuls, then tile-0 xab + wlh for its DoubleRow tail, then the
            # remaining L-slices in consumption order.
            def load_x(nt):
                nsl = slice(nt * NT, (nt + 1) * NT)
                nc.sync.dma_start(
                    x3_sb[:, :, nsl],
                    x3_dram.ap()[:, :, nsl].rearrange("k p l -> p k l"),
                )
                nc.sync.dma_start(
                    xab_sb[:, :, :, nsl],
                    xab_dram.ap()[:, :, :, nsl].rearrange("k a p l -> p k a l"),
                )

            nc.sync.dma_start(w3_sb[:], w3_dram.ap().rearrange("k p m -> p k m"))
            nsl0 = slice(0, NT)
            nc.sync.dma_start(
                x3_sb[:, :, nsl0], x3_dram.ap()[:, :, nsl0].rearrange("k p l -> p k l")
            )
            nc.sync.dma_start(
                xab_sb[:, :, :, nsl0],
                xab_dram.ap()[:, :, :, nsl0].rearrange("k a p l -> p k a l"),
            )
            nc.sync.dma_start(wlh_sb[:], wlh_dram.ap().rearrange("k a p m -> p k a m"))
            for nt in range(1, ntiles):
                load_x(nt)

            for nt in range(ntiles):
                nsl = slice(nt * NT, (nt + 1) * NT)
                o_sb = outp.tile([P, 2, NT], mybir.dt.bfloat16, tag="o")
                for m in range(2):
                    msl = slice(m * P, (m + 1) * P)
                    pt = ps.tile([P, NT], mybir.dt.float32, tag="ps")
                    # e3 part: plain fp8 matmuls over 4 chunks
                    for k in range(K3C):
                        nc.tensor.matmul(
                            pt[:],
                            w3_sb[:, k, msl],
                            x3_sb[:, k, nsl],
                            start=(k == 0),
                            stop=False,
                        )
                    # e4 bulk: wh @ xa over chunk pairs, DoubleRow
                    for q in range(K4C // 2):
                        nc.tensor.matmul(
                            pt[:],
                            wlh_sb[:, 2 * q:2 * q + 2, 1, msl],
                            xab_sb[:, 2 * q:2 * q + 2, 0, nsl],
                            start=False,
                            stop=False,
                            perf_mode=_DR,
                        )
                    # e4 corrections: wl@xa + wh@xb, paired per chunk
                    for i in range(K4C):
                        nc.tensor.matmul(
                            pt[:],
                            wlh_sb[:, i, :, msl],
                            xab_sb[:, i, :, nsl],
                            start=False,
                            stop=(i == K4C - 1),
                            perf_mode=_DR,
                        )
                    nc.any.tensor_copy(out=o_sb[:, m, :], in_=pt[:])
                nc.scalar.dma_start(
                    y_dram.ap()[:, :, nsl].rearrange("m p l -> p m l"), o_sb[:]
                )

    nc.finalize()
    return nc


_NC_CACHE = None


def kernel(x, W1, b1, W2, b2):
    global _NC_CACHE
    x = np.asarray(x)
    W1, b1 = np.asarray(W1), np.asarray(b1)
    W2, b2 = np.asarray(W2), np.asarray(b2)
    n, c, h, w = x.shape  # 4, 64, 512, 512

    # ---- host unfold: cols[b, c*16+kh*4+kw, ph*128+pw] = x[b,c,ph*4+kh,pw*4+kw]
    cols = x.reshape(n, c, 128, 4, 128, 4).transpose(0, 1, 3, 5, 2, 4)
    cols = np.ascontiguousarray(cols).reshape(n, 1024, 16384)

    # ---- fold the two GEMMs into one; quantize weights (shared scale 64)
    Weff = (W2.astype(np.float64) @ W1.astype(np.float64)).astype(np.float32)
    Wt64 = np.ascontiguousarray(Weff.T) * np.float32(64.0)  # [1024, 256]
    w3 = Wt64[:512].astype(_E3).reshape(K3C, P, COUT)
    wh = Wt64[512:].astype(_E4)
    wl = (Wt64[512:] - wh.astype(np.float32)).astype(_E4)
    wlh = np.ascontiguousarray(np.stack(
        [wl.reshape(K4C, P, COUT), wh.reshape(K4C, P, COUT)], axis=1
    ))

    if _NC_CACHE is None:
        _NC_CACHE = _build_nc()
    nc = _NC_CACHE

    in_maps = []
    for core in range(8):
        b, half = core // 2, core % 2
        cs = cols[b, :, half * LSH:(half + 1) * LSH] * np.float32(2.0)
        x3 = np.ascontiguousarray(cs[:512]).astype(_E3).reshape(K3C, P, LSH)
        x4 = np.ascontiguousarray(cs[512:])
        xa = x4.astype(_E4)
        xb = (x4 - xa.astype(np.float32)).astype(_E4)
        xab = np.ascontiguousarray(np.stack(
            [xa.reshape(K4C, P, LSH), xb.reshape(K4C, P, LSH)], axis=1
        ))
        in_maps.append({"x3": x3, "xab": xab, "w3": w3, "wlh": wlh})

    res = run_bass_kernel_spmd(nc, in_maps, core_ids=list(range(8)))

    # ---- gather + rescale (device computed 128*y) + fold on host
    y2 = np.empty((n, COUT, 16384), dtype=np.float32)
    for core in range(8):
        b, half = core // 2, core % 2
        y2[b, :, half * LSH:(half + 1) * LSH] = (
            res.results[core]["y"].reshape(COUT, LSH).astype(np.float32)
        )
    y2 *= np.float32(1.0 / 128.0)

    # bias epilogue (b1/b2 are zeros in this problem; exact otherwise)
    v = W2.astype(np.float64) @ b1.astype(np.float64) + b2.astype(np.float64)
    if np.any(v):
        y2 += v.astype(np.float32)[None, :, None]

    out = y2.reshape(n, c, 2, 2, 128, 128).transpose(0, 1, 4, 2, 5, 3)
    return np.ascontiguousarray(out).reshape(n, c, 256, 256)


# revision 41
# speedup vs baseline: 5.3632x; 1.1237x over previous
"""M2MRF module as a single fused mixed-precision GEMM on 8 TRN2 NeuronCores.

The reference chains two 1x1 convs with no nonlinearity between them, so
    y2 = W2 @ (W1 @ cols + b1) + b2 = W_eff @ cols + const,
with W_eff = W2 @ W1 precomputed on host (fp64). The device runs the
single GEMM y = W_eff @ cols per batch shard:
    cols = unfold(x[b], k=4, s=4)            # [1024, 16384]
    y    = W_eff @ cols                      # [256, 16384]
    out[b] = fold(y, k=2, s=2)               # [64, 256, 256]

Sharding: 8 cores = 4 batches x 2 L-halves (LSH = 8192 columns/core).

Precision scheme (all products share scale S = 128, one PSUM group):
 - channels 0..511  (e3 part): x3 = e3m4(2x), w3 = e3m4(64W); plain
   fp8 matmuls (1 cycle/row).
 - channels 512..895 (e4 part, 3 chunks): xa = e4m3(2x),
   xb = e4m3(2x - xa), wh = e4m3(64W), wl = e4m3(64W - wh). DoubleRow
   fp8 matmuls (0.5 cycle/row) compute wh@xa (bulk, chunk pairs) plus
   the correction wl@xa + wh@xb (paired per chunk), dropping only the
   tiny wl@xb term.
 - channels 896..1023 (1 chunk): bare e4m3 xa, no correction terms —
   trades quantization error for 1MB less DMA and 8% fewer PE cycles.
PSUM accumulates everything in fp32; result is written as bf16 of
128*y and rescaled on host. Measured rel err vs the fp32 reference is
1.72e-2 (gate: 2e-2), identical on host and device.
"""
import sys

sys.path.insert(0, "/opt/trn_rl_repo")

import numpy as np
import ml_dtypes

import concourse.bass as bass
import concourse.bacc as bacc
import concourse.mybir as mybir
import concourse.tile as tile
from concourse.bass_utils import run_bass_kernel_spmd

P = 128
NT = 512            # free-dim tile (one PSUM bank of fp32)
LSH = 8192          # L per core
NTILES = LSH // NT  # 16
K3C = 4             # e3m4 chunks (channels 0..511)
K4C = 4             # e4m3 chunks (channels 512..1023)
COUT = 256

_BF16 = ml_dtypes.bfloat16
_E3 = ml_dtypes.float8_e3m4
_E4 = ml_dtypes.float8_e4m3
_DR = mybir.MatmulPerfMode.DoubleRow


def _build_nc(ntiles=NTILES):
    nc = bacc.Bacc("TRN2", target_bir_lowering=False)
    x3_dram = nc.dram_tensor("x3", [K3C, P, LSH], mybir.dt.float8e3, kind="ExternalInput")
    # e4 chunks 0..2 ship (xa, xb) pairs; chunk 3 ships xa only (its wl/xb
    # corrections are dropped — measured rel err 1.72e-2 vs the 2e-2 gate)
    xab_dram = nc.dram_tensor("xab", [K4C - 1, 2, P, LSH], mybir.dt.float8e4, kind="ExternalInput")
    xr_dram = nc.dram_tensor("xr", [P, LSH], mybir.dt.float8e4, kind="ExternalInput")
    # weights are partition-major in DRAM: per-partition runs of 1-2KB keep
    # DMA descriptors above the 512B full-bandwidth threshold
    w3_dram = nc.dram_tensor("w3", [P, K3C, COUT], mybir.dt.float8e3, kind="ExternalInput")
    wlh_dram = nc.dram_tensor("wlh", [P, K4C, 2, COUT], mybir.dt.float8e4, kind="ExternalInput")
    y_dram = nc.dram_tensor("y", [2, P, LSH], mybir.dt.bfloat16, kind="ExternalOutput")

    with tile.TileContext(nc) as tc:
        with (
            tc.tile_pool(name="resident", bufs=1) as res,
            tc.tile_pool(name="outp", bufs=4) as outp,
            tc.tile_pool(name="ps", bufs=4, space="PSUM") as ps,
            tc.tile_pool(name="wps", bufs=1, space="PSUM") as wps,
        ):
            # PE p-state warmup: stream dummy matmuls on a zeroed tile while
            # the first DMAs are in flight, so real matmuls start at full
            # clock (the PE ramps over its first 3us of continuous activity).
            wu_sb = res.tile([P, 2, NT // 2], mybir.dt.float8e4, tag="wu")
            nc.vector.memset(wu_sb[:], 0)
            wu_pt = wps.tile([P, NT // 2], mybir.dt.float32, tag="wps")
            for _ in range(12):
                nc.tensor.matmul(
                    wu_pt[:],
                    wu_sb[:, :, 0:P],
                    wu_sb[:],
                    start=True,
                    stop=True,
                    perf_mode=_DR,
                )

            w3_sb = res.tile([P, K3C, COUT], mybir.dt.float8e3, tag="w3")
            wlh_sb = res.tile([P, K4C, 2, COUT], mybir.dt.float8e4, tag="wlh")
            x3_sb = res.tile([P, K3C, LSH], mybir.dt.float8e3, tag="x3")
            xab_sb = res.tile([P, K4C, 2, LSH], mybir.dt.float8e4, tag="xab")
            # DMA order tracks first use: w3 + tile-0 x3 feed the opening e3
            # matmuls, then tile-0 xab + wlh for its DoubleRow tail, then the
            # remaining L-slices in consumption order.
            # L-tiles: uniform 512 except the last 512 split in two, which
            # shortens the end-of-kernel copy+DMA tail.
            tiles = [(t * NT, NT) for t in range(ntiles - 1)]
            last = (ntiles - 1) * NT
            tiles += [(last, 256), (last + 256, 256)]

            def load_x(off, sz):
                nsl = slice(off, off + sz)
                nc.sync.dma_start(
                    x3_sb[:, :, nsl],
                    x3_dram.ap()[:, :, nsl].rearrange("k p l -> p k l"),
                )
                nc.sync.dma_start(
                    xab_sb[:, 0:K4C - 1, :, nsl],
                    xab_dram.ap()[:, :, :, nsl].rearrange("k a p l -> p k a l"),
                )

            def load_xr(off, sz):
                nsl = slice(off, off + sz)
                nc.sync.dma_start(xab_sb[:, K4C - 1, 0, nsl], xr_dram.ap()[:, nsl])

            # Input DMA stays on uniform 512-col slices (512B+ descriptor
            # runs avoid the sub-512B bandwidth penalty); the compute tiling
            # below may be finer — region tracking handles containment.
            # Order tracks first use: w3 + tile-0 x3 chunks feed the opening
            # e3 matmuls; wlh (small) lands before the bulkier tile-0 xab.
            nc.sync.dma_start(w3_sb[:], w3_dram.ap())
            nsl0 = slice(0, NT)
            nc.sync.dma_start(
                x3_sb[:, :, nsl0], x3_dram.ap()[:, :, nsl0].rearrange("k p l -> p k l")
            )
            nc.sync.dma_start(wlh_sb[:], wlh_dram.ap())
            nc.sync.dma_start(
                xab_sb[:, 0:K4C - 1, :, nsl0],
                xab_dram.ap()[:, :, :, nsl0].rearrange("k a p l -> p k a l"),
            )
            # xr (chunk-3 xa) in three coarse pieces to keep the DMA
            # instruction count (and its ~632ns HWDGE dispatch cost) low.
            load_xr(0, NT)
            load_x(NT, NT)
            load_xr(NT, 7 * NT)
            for nt in range(2, 9):
                load_x(nt * NT, NT)
            load_xr(8 * NT, 8 * NT)
            for nt in range(9, ntiles):
                load_x(nt * NT, NT)

            for off, sz in tiles:
                nsl = slice(off, off + sz)
                o_sb = outp.tile([P, 2, NT], mybir.dt.bfloat16, tag="o")
                for m in range(2):
                    msl = slice(m * P, (m + 1) * P)
                    pt = ps.tile([P, NT], mybir.dt.float32, tag="ps")
                    # e3 part: plain fp8 matmuls over 4 chunks
                    for k in range(K3C):
                        nc.tensor.matmul(
                            pt[:, :sz],
                            w3_sb[:, k, msl],
                            x3_sb[:, k, nsl],
                            start=(k == 0),
                            stop=False,
                        )
                    # e4 bulk: wh @ xa over chunk pairs, DoubleRow
                    for q in range(K4C // 2):
                        nc.tensor.matmul(
                            pt[:, :sz],
                            wlh_sb[:, 2 * q:2 * q + 2, 1, msl],
                            xab_sb[:, 2 * q:2 * q + 2, 0, nsl],
                            start=False,
                            stop=False,
                            perf_mode=_DR,
                        )
                    # e4 corrections: wl@xa + wh@xb, paired per chunk
                    # (chunk 3 ships xa only — no correction terms)
                    for i in range(K4C - 1):
                        nc.tensor.matmul(
                            pt[:, :sz],
                            wlh_sb[:, i, :, msl],
                            xab_sb[:, i, :, nsl],
                            start=False,
                            stop=(i == K4C - 2),
                            perf_mode=_DR,
                        )
                    nc.vector.tensor_copy(out=o_sb[:, m, :sz], in_=pt[:, :sz])
                    if sz != NT:
                        # tail tiles: per-m DMA shortens the final chain
                        nc.scalar.dma_start(y_dram.ap()[m][:, nsl], o_sb[:, m, :sz])
                if sz == NT:
                    # merged per-tile output DMA keeps instruction count low
                    nc.scalar.dma_start(
                        y_dram.ap()[:, :, nsl].rearrange("m p l -> p m l"),
                        o_sb[:],
                    )

    nc.finalize()
    return nc


_NC_CACHE = None


def kernel(x, W1, b1, W2, b2):
    global _NC_CACHE
    x = np.asarray(x)
    W1, b1 = np.asarray(W1), np.asarray(b1)
    W2, b2 = np.asarray(W2), np.asarray(b2)
    n, c, h, w = x.shape  # 4, 64, 512, 512

    # ---- host unfold: cols[b, c*16+kh*4+kw, ph*128+pw] = x[b,c,ph*4+kh,pw*4+kw]
    cols = x.reshape(n, c, 128, 4, 128, 4).transpose(0, 1, 3, 5, 2, 4)
    cols = np.ascontiguousarray(cols).reshape(n, 1024, 16384)

    # ---- fold the two GEMMs into one; quantize weights (shared scale 64)
    Weff = (W2.astype(np.float64) @ W1.astype(np.float64)).astype(np.float32)
    Wt64 = np.ascontiguousarray(Weff.T) * np.float32(64.0)  # [1024, 256]
    w3 = np.ascontiguousarray(
        Wt64[:512].astype(_E3).reshape(K3C, P, COUT).transpose(1, 0, 2)
    )  # [P, K3C, COUT]
    wh = Wt64[512:].astype(_E4)
    wl = (Wt64[512:] - wh.astype(np.float32)).astype(_E4)
    wlh = np.ascontiguousarray(np.stack(
        [wl.reshape(K4C, P, COUT), wh.reshape(K4C, P, COUT)], axis=1
    ).transpose(2, 0, 1, 3))  # [P, K4C, 2, COUT]

    if _NC_CACHE is None:
        _NC_CACHE = _build_nc()
    nc = _NC_CACHE

    in_maps = []
    for core in range(8):
        b, half = core // 2, core % 2
        cs = cols[b, :, half * LSH:(half + 1) * LSH] * np.float32(2.0)
        x3 = np.ascontiguousarray(cs[:512]).astype(_E3).reshape(K3C, P, LSH)
        x4 = np.ascontiguousarray(cs[512:])
        xa = x4.astype(_E4)
        xb = (x4[:384] - xa[:384].astype(np.float32)).astype(_E4)
        xab = np.ascontiguousarray(np.stack(
            [xa[:384].reshape(K4C - 1, P, LSH), xb.reshape(K4C - 1, P, LSH)], axis=1
        ))
        xr = np.ascontiguousarray(xa[384:])
        in_maps.append({"x3": x3, "xab": xab, "xr": xr, "w3": w3, "wlh": wlh})

    res = run_bass_kernel_spmd(nc, in_maps, core_ids=list(range(8)))

    # ---- gather + rescale (device computed 128*y) + fold on host
    y2 = np.empty((n, COUT, 16384), dtype=np.float32)
    for core in range(8):
        b, half = core // 2, core % 2
        y2[b, :, half * LSH:(half + 1) * LSH] = (
            res.results[core]["y"].reshape(COUT, LSH).astype(np.float32)
        )
    y2 *= np.float32(1.0 / 128.0)

    # bias epilogue (b1/b2 are zeros in this problem; exact otherwise)
    v = W2.astype(np.float64) @ b1.astype(np.float64) + b2.astype(np.float64)
    if np.any(v):
        y2 += v.astype(np.float32)[None, :, None]

    out = y2.reshape(n, c, 2, 2, 128, 128).transpose(0, 1, 4, 2, 5, 3)
    return np.ascontiguousarray(out).reshape(n, c, 256, 256)
